# revision 3
# baseline (speedup 1.0000x reference)
"""TRN2 Bass kernel v2 for nn_MultiHeadAttention_42511586296095.

Reference math (B=4, S=2048, E=768, H=12, full-width per-head projections):
    q_h = x @ Wq_h + bq_h ; k_h = x @ Wk_h + bk_h ; v_h = x @ Wv_h + bv_h
    attn_h = softmax(q_h k_h^T / 8)
    out = sum_h (attn_h v_h) @ W0_h + b0

Sharding: 8 cores = 4 batches x 2 head-groups (6 heads each). Host sums the
two per-batch partials and adds constants.

v2 restructure (vs the v1 baseline):
  - W0 folded into the V projection: C_h = Wv_h @ W0_h, so PV directly
    produces output features. Removes the separate per-head W0 pass
    (12% of MACs) and the bv contribution becomes a host constant.
  - Transposed PV: the exp-strip chunk [j,128i] is the STATIONARY operand and
    vW [j, f] the moving one, so PV output lands as [i (partitions), f] —
    queries on partitions. Softmax normalization is then a per-partition
    scale: one fused DVE scalar_tensor_tensor (psum*recip + out_acc) per
    output tile, and no final transposes / host head-sum.
  - Denominator for free: vWaug col 0 is all-ones, so PV's first column
    accumulates sum_j exp(...) alongside the output features.
  - Strips and vW in bf16 (full-rate PE, 1024-wide moving operands): PV is
    2 matmuls per (jt, i-tile): 512-wide (ones+511 f-cols) + 257-wide.
  - Scores path stays f32r (exp amplifies score errors; bf16 only after exp).

Per core per head (f32r/bf16 matmuls = full-rate PE):
    P1: uT[f,j] = sum_e A[e,f] xT[e,j]          (A = Wk Wq^T, host)
    P2: vW[j,f] = sum_e x[j,e] C[e,f] -> bf16   (C = Wv W0, host)
    P3 per 256-wide i-group: per j-tile: scoresT psum -> ACT exp
        (scale=1/8, bias=beta/8) -> bf16 strip; 2 PV matmuls per i-tile
        accumulate [i, 1+768] in PSUM across j-tiles.
    Normalize+accumulate: oac[i-tile] (+)= py[:,1:] * recip(py[:,0]) on DVE.
"""

import numpy as np

import concourse.bass as bass
import concourse.mybir as mybir
import concourse.tile as tile
from concourse import bacc
from concourse.bass_utils import run_bass_kernel_spmd

F32 = mybir.dt.float32
F32R = mybir.dt.float32r
BF16 = mybir.dt.bfloat16
EXP = mybir.ActivationFunctionType.Exp
MUL = mybir.AluOpType.mult
ADD = mybir.AluOpType.add

B, S, E, H = 4, 2048, 768, 12
HPC = 6          # heads per core
EC = E // 128    # 6 chunks of the feature dim
JT = S // 128    # 16 key tiles
IG = 8           # query groups
IGW = S // IG    # 256 queries per group
ITS = IGW // 128 # 2 i-tiles per group
NT = S // 128    # 16 output row tiles
FGW = 256        # C slice width for P2
FG = E // FGW    # 3 slices
VW = E + 1       # vW width incl. ones column

_CACHED_NC = None


def _round_f32r(x: np.ndarray) -> np.ndarray:
    """Round fp32 to the hw f32r format: 11 explicit mantissa bits, RNE."""
    b = np.ascontiguousarray(x, dtype=np.float32).view(np.uint32).astype(np.uint64)
    shift = 12
    half = np.uint64(1 << (shift - 1))
    mask = np.uint64((1 << shift) - 1)
    r = (b + half) & ~mask
    tie = (b & mask) == half
    r[tie] = (b[tie] & ~mask) + (
        ((b[tie] >> np.uint64(shift)) & np.uint64(1)) << np.uint64(shift)
    )
    return r.astype(np.uint32).view(np.float32).reshape(x.shape)


def _chunked(a: np.ndarray) -> np.ndarray:
    """[E, N] -> SBUF layout [128, EC, N] with e = ec*128 + p."""
    ec = a.shape[0] // 128
    return np.ascontiguousarray(a.reshape(ec, 128, -1).transpose(1, 0, 2))


def _build_nc(hpc=HPC, loop=None):
    nc = bacc.Bacc("TRN2", target_bir_lowering=False, debug=False, num_devices=8)

    xT_d = nc.dram_tensor("xT", [4, 128, EC, 512], F32R, kind="ExternalInput")
    A_d = nc.dram_tensor("A", [HPC, EC, 128, EC, 128], F32R, kind="ExternalInput")
    C_d = nc.dram_tensor("C", [HPC, 128, EC, E], F32R, kind="ExternalInput")
    beta_d = nc.dram_tensor("beta8", [HPC, 128, JT], F32, kind="ExternalInput")
    out_d = nc.dram_tensor("out", [S, E], F32, kind="ExternalOutput")

    with tile.TileContext(nc) as tc:
        with (
            tc.tile_pool(name="big", bufs=1) as big,
            tc.tile_pool(name="wts", bufs=1) as wts,
            tc.tile_pool(name="strips", bufs=3) as strips_p,
            tc.tile_pool(name="small", bufs=1) as small,
            tc.tile_pool(name="ps", bufs=1, space="PSUM") as ps,
        ):
            xT = big.tile([128, EC, S], F32R, name="xT_sb")
            for c in range(4):
                nc.sync.dma_start(xT[:, :, c * 512:(c + 1) * 512], xT_d.ap()[c])
            uT = big.tile([128, EC, S], F32R, name="uT_sb")
            vW = big.tile([128, JT, VW], BF16, name="vW_sb")
            nc.vector.memset(vW[:, :, 0:1], 1.0)
            oac = big.tile([128, NT, E], F32, name="oac_sb")

            import contextlib
            loop_cm = tc.For_i(0, loop, 1) if loop else contextlib.nullcontext()
            with loop_cm:
              for h in range(hpc):
                  beta_sb = wts.tile([128, JT], F32, tag="beta", bufs=2,
                                     name=f"beta_{h}")
                  nc.sync.dma_start(beta_sb[:], beta_d.ap()[h])

                  # ---- P1 (uT) and P2 (vW) interleaved, stationary-reuse
                  # order: consecutive matmuls share the same lhsT so the
                  # f32r weight reload is skipped (hw fast path) ----
                  a_sl = {}

                  def load_a(fc, h=h):
                      t = wts.tile([128, EC, 128], F32R, tag="a_sl", bufs=2,
                                   name=f"a_{h}_{fc}")
                      nc.sync.dma_start(t[:], A_d.ap()[h][fc])
                      return t

                  a_sl[0] = load_a(0)
                  C_sb = wts.tile([128, EC, E], F32R, tag="c", bufs=1,
                                  name=f"c_{h}")
                  nc.sync.dma_start(C_sb[:], C_d.ap()[h])

                  def p1_unit(fc, h=h):
                      if fc + 1 < EC and fc + 1 not in a_sl:
                          a_sl[fc + 1] = load_a(fc + 1)
                      for bb in range(2):
                          pu = [
                              ps.tile([128, 512], F32, tag=tg, bufs=nb,
                                      name=f"pu_{h}_{fc}_{bb}_{j}")
                              for j, (tg, nb) in enumerate(
                                  [("ya", 3), ("sc", 2)]
                              )
                          ]
                          for ec in range(EC):
                              for j in range(2):
                                  jg = bb * 2 + j
                                  nc.tensor.matmul(
                                      pu[j][:], a_sl[fc][:, ec, :],
                                      xT[:, ec, jg * 512:(jg + 1) * 512],
                                      start=(ec == 0), stop=(ec == EC - 1),
                                  )
                          for j in range(2):
                              jg = bb * 2 + j
                              nc.vector.tensor_copy(
                                  uT[:, fc, jg * 512:(jg + 1) * 512], pu[j][:],
                              )

                  def p2_unit(jt, h=h):
                      pv = [
                          ps.tile([128, 257], F32, tag="yb", bufs=3,
                                  name=f"pv_{h}_{jt}_{fg}")
                          for fg in range(FG)
                      ]
                      for ec in range(EC):
                          for fg in range(FG):
                              nc.tensor.matmul(
                                  pv[fg][:, 0:FGW],
                                  xT[:, ec, jt * 128:(jt + 1) * 128],
                                  C_sb[:, ec, fg * FGW:(fg + 1) * FGW],
                                  start=(ec == 0), stop=(ec == EC - 1),
                              )
                      for fg in range(FG):
                          nc.vector.tensor_copy(
                              vW[:, jt, 1 + fg * FGW:1 + (fg + 1) * FGW],
                              pv[fg][:, 0:FGW],
                          )

                  p2n = [3, 3, 2, 3, 3, 2]
                  jt_i = 0
                  for fc in range(EC):
                      p1_unit(fc)
                      for _ in range(p2n[fc]):
                          p2_unit(jt_i)
                          jt_i += 1
                  assert jt_i == JT

                  # ---- P3 per i-group: scores -> exp -> PV -> normalize ----
                  for ig in range(IG):
                      isl = slice(ig * IGW, (ig + 1) * IGW)
                      pya = [
                          ps.tile([128, 512], F32, tag="ya", bufs=3,
                                  name=f"pya_{h}_{ig}_{it}")
                          for it in range(ITS)
                      ]
                      pyb = [
                          ps.tile([128, 257], F32, tag="yb", bufs=3,
                                  name=f"pyb_{h}_{ig}_{it}")
                          for it in range(ITS)
                      ]
                      strips = {}

                      def scores_strip(jt, h=h, ig=ig, isl=isl):
                          psct = ps.tile([128, 512], F32, tag="sc", bufs=2,
                                         name=f"ps_{h}_{ig}_{jt}")
                          psc = psct[:, 0:IGW]
                          for fc in range(EC):
                              nc.tensor.matmul(
                                  psc, uT[:, fc, jt * 128:(jt + 1) * 128],
                                  xT[:, fc, isl],
                                  start=(fc == 0), stop=(fc == EC - 1),
                              )
                          st = strips_p.tile([128, IGW], BF16, tag="s",
                                             name=f"st_{h}_{ig}_{jt}")
                          nc.scalar.activation(
                              st[:], psc, EXP,
                              bias=beta_sb[:, jt:jt + 1], scale=0.125,
                          )
                          strips[jt] = st

                      def pv_strip(jt, pya=pya, pyb=pyb):
                          st = strips.pop(jt)
                          for it in range(ITS):
                              stc = st[:, it * 128:(it + 1) * 128]
                              nc.tensor.matmul(
                                  pya[it][:], stc, vW[:, jt, 0:512],
                                  start=(jt == 0), stop=(jt == JT - 1),
                              )
                              nc.tensor.matmul(
                                  pyb[it][:], stc, vW[:, jt, 512:VW],
                                  start=(jt == 0), stop=(jt == JT - 1),
                              )

                      scores_strip(0)
                      for jt in range(1, JT):
                          scores_strip(jt)
                          pv_strip(jt - 1)
                      pv_strip(JT - 1)

                      for it in range(ITS):
                          t = ig * ITS + it
                          rc = small.tile([128, 1], F32, tag="rc", bufs=2,
                                          name=f"rc_{h}_{ig}_{it}")
                          nc.vector.reciprocal(rc[:], pya[it][:, 0:1])
                          if h == 0:
                              nc.vector.tensor_scalar_mul(
                                  oac[:, t, 0:511], pya[it][:, 1:512], rc[:],
                              )
                              nc.vector.tensor_scalar_mul(
                                  oac[:, t, 511:E], pyb[it][:, 0:257], rc[:],
                              )
                          else:
                              nc.vector.scalar_tensor_tensor(
                                  oac[:, t, 0:511], pya[it][:, 1:512], rc[:],
                                  oac[:, t, 0:511], op0=MUL, op1=ADD,
                              )
                              nc.vector.scalar_tensor_tensor(
                                  oac[:, t, 511:E], pyb[it][:, 0:257], rc[:],
                                  oac[:, t, 511:E], op0=MUL, op1=ADD,
                              )
                          if h == hpc - 1:
                              nc.gpsimd.dma_start(
                                  out_d.ap()[t * 128:(t + 1) * 128, :],
                                  oac[:, t, :],
                              )

    nc.compile()
    return nc


def _get_nc():
    global _CACHED_NC
    if _CACHED_NC is None:
        _CACHED_NC = _build_nc()
    return _CACHED_NC


def _prepare_inputs(x, Wq, Wk, Wv, bq, bk, bv, W0, b0):
    x = np.asarray(x, dtype=np.float32)
    Wq = np.asarray(Wq, dtype=np.float32)
    Wk = np.asarray(Wk, dtype=np.float32)
    Wv = np.asarray(Wv, dtype=np.float32)
    bq = np.asarray(bq, dtype=np.float32)
    bv = np.asarray(bv, dtype=np.float32)
    W0 = np.asarray(W0, dtype=np.float32)
    b0 = np.asarray(b0, dtype=np.float32)

    # Per-head host precomputation (shared across batches)
    A = np.matmul(Wk, Wq.transpose(0, 2, 1))          # [H, E, E] = Wk @ Wq^T
    W0h = W0.reshape(H, E, E)
    C = np.matmul(Wv, W0h)                            # [H, E, E] folded V*W0
    wbeta = np.einsum("hef,hf->he", Wk, bq)           # [H, E]
    b_eff = b0 + np.einsum("he,hen->n", bv, W0h)      # [E]

    # A as 6 f-slices of [128, EC, 128]; C whole-chunked per head
    A_sl = np.empty((H, EC, 128, EC, 128), dtype=np.float32)
    C_sl = np.empty((H, 128, EC, E), dtype=np.float32)
    for h in range(H):
        Ac = _chunked(_round_f32r(A[h]))
        C_sl[h] = _chunked(_round_f32r(C[h]))
        for fc in range(EC):
            A_sl[h, fc] = Ac[:, :, fc * 128:(fc + 1) * 128]

    in_maps = []
    for c in range(8):
        b, hg = divmod(c, 2)
        hs = hg * HPC
        xTc = _chunked(_round_f32r(x[b].T))            # [128, EC, S]
        xT4 = np.stack(
            [xTc[:, :, i * 512:(i + 1) * 512] for i in range(4)]
        )                                              # [4, 128, EC, 512]
        beta8 = np.einsum("se,he->hs", x[b], wbeta[hs:hs + HPC]) / 8.0
        beta8 = np.ascontiguousarray(
            beta8.reshape(HPC, JT, 128).transpose(0, 2, 1), dtype=np.float32
        )                                              # [HPC, 128, JT]
        in_maps.append({
            "xT": np.ascontiguousarray(xT4),
            "A": np.ascontiguousarray(A_sl[hs:hs + HPC]),
            "C": np.ascontiguousarray(C_sl[hs:hs + HPC]),
            "beta8": beta8,
        })
    return in_maps, b_eff


def kernel(x, Wq, Wk, Wv, bq, bk, bv, W0, b0, _return_results=False):
    in_maps, b_eff = _prepare_inputs(x, Wq, Wk, Wv, bq, bk, bv, W0, b0)
    nc = _get_nc()
    res = run_bass_kernel_spmd(nc, in_maps, core_ids=list(range(8)))
    out = np.zeros((B, S, E), dtype=np.float32)
    for c in range(8):
        out[c // 2] += res.results[c]["out"]
    out += b_eff[None, None, :]
    if _return_results:
        return out, res
    return out


# revision 4
# speedup vs baseline: 2.9349x; 2.9349x over previous
"""TRN2 Bass kernel v2 for nn_MultiHeadAttention_42511586296095.

Reference math (B=4, S=2048, E=768, H=12, full-width per-head projections):
    q_h = x @ Wq_h + bq_h ; k_h = x @ Wk_h + bk_h ; v_h = x @ Wv_h + bv_h
    attn_h = softmax(q_h k_h^T / 8)
    out = sum_h (attn_h v_h) @ W0_h + b0

Sharding: 8 cores = 4 batches x 2 head-groups (6 heads each). Host sums the
two per-batch partials and adds constants.

v2 restructure (vs the v1 baseline):
  - W0 folded into the V projection: C_h = Wv_h @ W0_h, so PV directly
    produces output features. Removes the separate per-head W0 pass
    (12% of MACs) and the bv contribution becomes a host constant.
  - Transposed PV: the exp-strip chunk [j,128i] is the STATIONARY operand and
    vW [j, f] the moving one, so PV output lands as [i (partitions), f] —
    queries on partitions. Softmax normalization is then a per-partition
    scale: one fused DVE scalar_tensor_tensor (psum*recip + out_acc) per
    output tile, and no final transposes / host head-sum.
  - Denominator for free: vWaug col 0 is all-ones, so PV's first column
    accumulates sum_j exp(...) alongside the output features.
  - Strips and vW in bf16 (full-rate PE, 1024-wide moving operands): PV is
    2 matmuls per (jt, i-tile): 512-wide (ones+511 f-cols) + 257-wide.
  - Scores path stays f32r (exp amplifies score errors; bf16 only after exp).

Per core per head (f32r/bf16 matmuls = full-rate PE):
    P1: uT[f,j] = sum_e A[e,f] xT[e,j]          (A = Wk Wq^T, host)
    P2: vW[j,f] = sum_e x[j,e] C[e,f] -> bf16   (C = Wv W0, host)
    P3 per 256-wide i-group: per j-tile: scoresT psum -> ACT exp
        (scale=1/8, bias=beta/8) -> bf16 strip; 2 PV matmuls per i-tile
        accumulate [i, 1+768] in PSUM across j-tiles.
    Normalize+accumulate: oac[i-tile] (+)= py[:,1:] * recip(py[:,0]) on DVE.
"""

import numpy as np

import concourse.bass as bass
import concourse.mybir as mybir
import concourse.tile as tile
from concourse import bacc
from concourse.bass_utils import run_bass_kernel_spmd

F32 = mybir.dt.float32
F32R = mybir.dt.float32r
BF16 = mybir.dt.bfloat16
EXP = mybir.ActivationFunctionType.Exp
MUL = mybir.AluOpType.mult
ADD = mybir.AluOpType.add

B, S, E, H = 4, 2048, 768, 12
HPC = 6          # heads per core
EC = E // 128    # 6 chunks of the feature dim
JT = S // 128    # 16 key tiles
IG = 8           # query groups
IGW = S // IG    # 256 queries per group
ITS = IGW // 128 # 2 i-tiles per group
NT = S // 128    # 16 output row tiles
FGW = 256        # C slice width for P2
FG = E // FGW    # 3 slices
VW = E + 1       # vW width incl. ones column

_CACHED_NC = None


def _round_f32r(x: np.ndarray) -> np.ndarray:
    """Round fp32 to the hw f32r format: 11 explicit mantissa bits, RNE."""
    b = np.ascontiguousarray(x, dtype=np.float32).view(np.uint32).astype(np.uint64)
    shift = 12
    half = np.uint64(1 << (shift - 1))
    mask = np.uint64((1 << shift) - 1)
    r = (b + half) & ~mask
    tie = (b & mask) == half
    r[tie] = (b[tie] & ~mask) + (
        ((b[tie] >> np.uint64(shift)) & np.uint64(1)) << np.uint64(shift)
    )
    return r.astype(np.uint32).view(np.float32).reshape(x.shape)


def _chunked(a: np.ndarray) -> np.ndarray:
    """[E, N] -> SBUF layout [128, EC, N] with e = ec*128 + p."""
    ec = a.shape[0] // 128
    return np.ascontiguousarray(a.reshape(ec, 128, -1).transpose(1, 0, 2))


def _build_nc(hpc=HPC, loop=None):
    nc = bacc.Bacc("TRN2", target_bir_lowering=False, debug=False, num_devices=8)

    xT_d = nc.dram_tensor("xT", [4, 128, EC, 512], F32R, kind="ExternalInput")
    A_d = nc.dram_tensor("A", [HPC, EC, 128, EC, 128], F32R, kind="ExternalInput")
    C_d = nc.dram_tensor("C", [HPC, 128, EC, E], F32R, kind="ExternalInput")
    beta_d = nc.dram_tensor("beta8", [HPC, 128, JT], F32, kind="ExternalInput")
    out_d = nc.dram_tensor("out", [S, E], F32, kind="ExternalOutput")

    with tile.TileContext(nc) as tc:
        with (
            tc.tile_pool(name="big", bufs=1) as big,
            tc.tile_pool(name="wts", bufs=1) as wts,
            tc.tile_pool(name="strips", bufs=3) as strips_p,
            tc.tile_pool(name="small", bufs=1) as small,
            tc.tile_pool(name="ps", bufs=1, space="PSUM") as ps,
        ):
            xT = big.tile([128, EC, S], F32R, name="xT_sb")
            for c in range(4):
                nc.sync.dma_start(xT[:, :, c * 512:(c + 1) * 512], xT_d.ap()[c])
            uT = big.tile([128, EC, S], F32R, name="uT_sb")
            vW = big.tile([128, JT, VW], BF16, name="vW_sb")
            nc.vector.memset(vW[:, :, 0:1], 1.0)
            oac = big.tile([128, NT, E], F32, name="oac_sb")

            import contextlib
            loop_cm = tc.For_i(0, loop, 1) if loop else contextlib.nullcontext()
            with loop_cm:
              for h in range(hpc):
                  beta_sb = wts.tile([128, JT], F32, tag="beta", bufs=2,
                                     name=f"beta_{h}")
                  nc.sync.dma_start(beta_sb[:], beta_d.ap()[h])

                  # ---- P1 (uT) and P2 (vW) interleaved, stationary-reuse
                  # order: consecutive matmuls share the same lhsT so the
                  # f32r weight reload is skipped (hw fast path) ----
                  a_sl = {}

                  def load_a(fc, h=h):
                      t = wts.tile([128, EC, 128], F32R, tag="a_sl", bufs=2,
                                   name=f"a_{h}_{fc}")
                      nc.sync.dma_start(t[:], A_d.ap()[h][fc])
                      return t

                  a_sl[0] = load_a(0)
                  C_sb = wts.tile([128, EC, E], F32R, tag="c", bufs=1,
                                  name=f"c_{h}")
                  nc.sync.dma_start(C_sb[:], C_d.ap()[h])

                  def p1_unit(fc, h=h):
                      if fc + 1 < EC and fc + 1 not in a_sl:
                          a_sl[fc + 1] = load_a(fc + 1)
                      for bb in range(2):
                          pu = [
                              ps.tile([128, 512], F32, tag=tg, bufs=nb,
                                      name=f"pu_{h}_{fc}_{bb}_{j}")
                              for j, (tg, nb) in enumerate(
                                  [("ya", 3), ("sc", 2)]
                              )
                          ]
                          for ec in range(EC):
                              for j in range(2):
                                  jg = bb * 2 + j
                                  nc.tensor.matmul(
                                      pu[j][:], a_sl[fc][:, ec, :],
                                      xT[:, ec, jg * 512:(jg + 1) * 512],
                                      start=(ec == 0), stop=(ec == EC - 1),
                                  )
                          for j in range(2):
                              jg = bb * 2 + j
                              nc.vector.tensor_copy(
                                  uT[:, fc, jg * 512:(jg + 1) * 512], pu[j][:],
                              )

                  def p2_unit(jt, h=h):
                      pvA = ps.tile([128, 512], F32, tag="ya", bufs=3,
                                    name=f"pva_{h}_{jt}")
                      pvB = ps.tile([128, 257], F32, tag="yb", bufs=3,
                                    name=f"pvb_{h}_{jt}")
                      for ec in range(EC):
                          nc.tensor.matmul(
                              pvA[:], xT[:, ec, jt * 128:(jt + 1) * 128],
                              C_sb[:, ec, 0:512],
                              start=(ec == 0), stop=(ec == EC - 1),
                          )
                          nc.tensor.matmul(
                              pvB[:, 0:FGW],
                              xT[:, ec, jt * 128:(jt + 1) * 128],
                              C_sb[:, ec, 512:E],
                              start=(ec == 0), stop=(ec == EC - 1),
                          )
                      nc.vector.tensor_copy(vW[:, jt, 1:513], pvA[:])
                      nc.vector.tensor_copy(vW[:, jt, 513:VW], pvB[:, 0:FGW])

                  p2n = [3, 3, 2, 3, 3, 2]
                  jt_i = 0
                  for fc in range(EC):
                      p1_unit(fc)
                      for _ in range(p2n[fc]):
                          p2_unit(jt_i)
                          jt_i += 1
                  assert jt_i == JT

                  # ---- P3 per i-group: scores -> exp -> PV -> normalize ----
                  for ig in range(IG):
                      isl = slice(ig * IGW, (ig + 1) * IGW)
                      pya = [
                          ps.tile([128, 512], F32, tag="ya", bufs=3,
                                  name=f"pya_{h}_{ig}_{it}")
                          for it in range(ITS)
                      ]
                      pyb = [
                          ps.tile([128, 257], F32, tag="yb", bufs=3,
                                  name=f"pyb_{h}_{ig}_{it}")
                          for it in range(ITS)
                      ]
                      strips = {}

                      def scores_strip(jt, h=h, ig=ig, isl=isl):
                          psct = ps.tile([128, 512], F32, tag="sc", bufs=2,
                                         name=f"ps_{h}_{ig}_{jt}")
                          psc = psct[:, 0:IGW]
                          for fc in range(EC):
                              nc.tensor.matmul(
                                  psc, uT[:, fc, jt * 128:(jt + 1) * 128],
                                  xT[:, fc, isl],
                                  start=(fc == 0), stop=(fc == EC - 1),
                              )
                          st = strips_p.tile([128, IGW], BF16, tag="s",
                                             name=f"st_{h}_{ig}_{jt}")
                          nc.scalar.activation(
                              st[:], psc, EXP,
                              bias=beta_sb[:, jt:jt + 1], scale=0.125,
                          )
                          strips[jt] = st

                      def pv_strip(jt, pya=pya, pyb=pyb):
                          st = strips.pop(jt)
                          for it in range(ITS):
                              stc = st[:, it * 128:(it + 1) * 128]
                              nc.tensor.matmul(
                                  pya[it][:], stc, vW[:, jt, 0:512],
                                  start=(jt == 0), stop=(jt == JT - 1),
                              )
                              nc.tensor.matmul(
                                  pyb[it][:], stc, vW[:, jt, 512:VW],
                                  start=(jt == 0), stop=(jt == JT - 1),
                              )

                      scores_strip(0)
                      for jt in range(1, JT):
                          scores_strip(jt)
                          pv_strip(jt - 1)
                      pv_strip(JT - 1)

                      for it in range(ITS):
                          t = ig * ITS + it
                          rc = small.tile([128, 1], F32, tag="rc", bufs=2,
                                          name=f"rc_{h}_{ig}_{it}")
                          nc.vector.reciprocal(rc[:], pya[it][:, 0:1])
                          if h == 0:
                              nc.vector.tensor_scalar_mul(
                                  oac[:, t, 0:511], pya[it][:, 1:512], rc[:],
                              )
                              nc.vector.tensor_scalar_mul(
                                  oac[:, t, 511:E], pyb[it][:, 0:257], rc[:],
                              )
                          else:
                              nc.vector.scalar_tensor_tensor(
                                  oac[:, t, 0:511], pya[it][:, 1:512], rc[:],
                                  oac[:, t, 0:511], op0=MUL, op1=ADD,
                              )
                              nc.vector.scalar_tensor_tensor(
                                  oac[:, t, 511:E], pyb[it][:, 0:257], rc[:],
                                  oac[:, t, 511:E], op0=MUL, op1=ADD,
                              )
                          if h == hpc - 1:
                              nc.gpsimd.dma_start(
                                  out_d.ap()[t * 128:(t + 1) * 128, :],
                                  oac[:, t, :],
                              )

    nc.compile()
    return nc


def _get_nc():
    global _CACHED_NC
    if _CACHED_NC is None:
        _CACHED_NC = _build_nc()
    return _CACHED_NC


def _prepare_inputs(x, Wq, Wk, Wv, bq, bk, bv, W0, b0):
    x = np.asarray(x, dtype=np.float32)
    Wq = np.asarray(Wq, dtype=np.float32)
    Wk = np.asarray(Wk, dtype=np.float32)
    Wv = np.asarray(Wv, dtype=np.float32)
    bq = np.asarray(bq, dtype=np.float32)
    bv = np.asarray(bv, dtype=np.float32)
    W0 = np.asarray(W0, dtype=np.float32)
    b0 = np.asarray(b0, dtype=np.float32)

    # Per-head host precomputation (shared across batches)
    A = np.matmul(Wk, Wq.transpose(0, 2, 1))          # [H, E, E] = Wk @ Wq^T
    W0h = W0.reshape(H, E, E)
    C = np.matmul(Wv, W0h)                            # [H, E, E] folded V*W0
    wbeta = np.einsum("hef,hf->he", Wk, bq)           # [H, E]
    b_eff = b0 + np.einsum("he,hen->n", bv, W0h)      # [E]

    # A as 6 f-slices of [128, EC, 128]; C whole-chunked per head
    A_sl = np.empty((H, EC, 128, EC, 128), dtype=np.float32)
    C_sl = np.empty((H, 128, EC, E), dtype=np.float32)
    for h in range(H):
        Ac = _chunked(_round_f32r(A[h]))
        C_sl[h] = _chunked(_round_f32r(C[h]))
        for fc in range(EC):
            A_sl[h, fc] = Ac[:, :, fc * 128:(fc + 1) * 128]

    in_maps = []
    for c in range(8):
        b, hg = divmod(c, 2)
        hs = hg * HPC
        xTc = _chunked(_round_f32r(x[b].T))            # [128, EC, S]
        xT4 = np.stack(
            [xTc[:, :, i * 512:(i + 1) * 512] for i in range(4)]
        )                                              # [4, 128, EC, 512]
        beta8 = np.einsum("se,he->hs", x[b], wbeta[hs:hs + HPC]) / 8.0
        beta8 = np.ascontiguousarray(
            beta8.reshape(HPC, JT, 128).transpose(0, 2, 1), dtype=np.float32
        )                                              # [HPC, 128, JT]
        in_maps.append({
            "xT": np.ascontiguousarray(xT4),
            "A": np.ascontiguousarray(A_sl[hs:hs + HPC]),
            "C": np.ascontiguousarray(C_sl[hs:hs + HPC]),
            "beta8": beta8,
        })
    return in_maps, b_eff


def kernel(x, Wq, Wk, Wv, bq, bk, bv, W0, b0, _return_results=False):
    in_maps, b_eff = _prepare_inputs(x, Wq, Wk, Wv, bq, bk, bv, W0, b0)
    nc = _get_nc()
    res = run_bass_kernel_spmd(nc, in_maps, core_ids=list(range(8)))
    out = np.zeros((B, S, E), dtype=np.float32)
    for c in range(8):
        out[c // 2] += res.results[c]["out"]
    out += b_eff[None, None, :]
    if _return_results:
        return out, res
    return out
